# revision 36
# baseline (speedup 1.0000x reference)
"""PINN Navier-Stokes residual kernel for trn2 (8 cores, data parallel).

13-stream Taylor jet {v,x,y,t,xx,xy,yy,xt,yt,xxx,xxy,xyy,yyy} through the
3-128x8-2 tanh MLP.  Activations are [128 hidden, Npts] tiles; each hidden
layer is 28 fp16 matmuls per 512-pt chunk (addends summed in PSUM).

v3 engine plan:
 - chain-rule constants (-2/-4/-6) folded into pre-scaled stationary
   weight copies (zero extra cost on PE, kills all elementwise scaling)
 - layer0's input-jet constants folded into 13 per-stream layer1
   stationaries, so layer0 emits only {s, s1, m=s*s1, s3h}
 - elementwise products grouped into wide chunk-major DVE tensor_tensor
   lumps via stride-0 broadcast APs
 - the 6 per-chunk PSUM tail products (s1*z) run on GpSimd
 - Act: tanh, t1, s1, PSUM->SBUF copies, squares
"""

import os
import numpy as np
from contextlib import ExitStack

import concourse.bass as bass
import concourse.bacc as bacc
import concourse.tile as tile
from concourse import mybir
from concourse.bass_utils import run_bass_kernel_spmd

F32 = mybir.dt.float32
F16 = mybir.dt.float16
OP = mybir.AluOpType
AF = mybir.ActivationFunctionType

N_CORES = 8
N = 32768
NLOC = N // N_CORES      # 4096 points per core
BLK = 1024               # elementwise batch
NBLK = NLOC // BLK       # 4
CH = 512                 # matmul / psum chunk
NCH = BLK // CH          # 2
H = 128
PB = NLOC // H           # 32 free cols in final [128, PB] tiles

STREAMS = ["v", "x", "y", "t", "xx", "xy", "yy", "xt", "yt",
           "xxx", "xxy", "xyy", "yyy"]
ZCOPY = ["x", "y", "t", "xx", "xy", "yy"]
ZSEG = {s: i for i, s in enumerate(ZCOPY)}
TAILS = ["xt", "yt", "xxx", "xxy", "xyy", "yyy"]

# addend table: stream -> list of (tile_key, seg_index, weight_variant)
# weight_variant 0 -> W, 2 -> -2W, 4 -> -4W, 6 -> -6W
ADDENDS = {
    "v":   [("s", 0, 0)],
    "x":   [("G1", 0, 0)],
    "y":   [("G1", 1, 0)],
    "t":   [("G1", 2, 0)],
    "xx":  [("EXG", 0, 2), ("G1", 3, 0)],
    "xy":  [("EXG", 1, 2), ("G1", 4, 0)],
    "yy":  [("EYG", 0, 2), ("G1", 5, 0)],
    "xt":  [("EXG", 2, 2), ("Txt", 0, 0)],
    "yt":  [("EYG", 1, 2), ("Tyt", 0, 0)],
    "xxx": [("FG", 0, 0), ("EXG", 3, 6), ("Txxx", 0, 0)],
    "xxy": [("FG", 1, 0), ("EYG", 2, 2), ("EXG", 4, 4), ("Txxy", 0, 0)],
    "xyy": [("FG", 2, 0), ("EXG", 5, 2), ("EYG", 3, 4), ("Txyy", 0, 0)],
    "yyy": [("FG", 3, 0), ("EYG", 4, 6), ("Tyyy", 0, 0)],
}
# segments per chunk for each addend-group tile (chunk-major layout)
GROUP_SEGS = {"s": 1, "G1": 6, "EXG": 6, "EYG": 5, "FG": 4,
              "Txt": 1, "Tyt": 1, "Txxx": 1, "Txxy": 1, "Txyy": 1,
              "Tyyy": 1}
VARIANTS = [0, 2, 4, 6]

# engine assignment knobs: 'V' = Vector, 'S' = Scalar/Act, 'G' = GpSimd,
# 'D' = DMA-stage the PSUM tile to SBUF f32, then multiply on GpSimd.
# GpSimd cannot read PSUM directly on real HW.
ENG_TAIL = "V"
ENG_PCAT = "S"      # [zx^2|zy^2]
ENG_E = "G"         # [ex|ey] = m * [zx|zy]
ENG_F = "G"         # [fx|fy] = s3h * pcat
ENG_FG = "G"        # [fx|fx|fy|fy] * [zx|zy|zx|zy]
# layer0 -> layer1: moving tensor per stream (constants folded into W1_s)
L1_MOV = {"v": "s", "x": "s1", "y": "s1", "t": "s1",
          "xx": "m", "xy": "m", "yy": "m", "xt": "m", "yt": "m",
          "xxx": "s3h", "xxy": "s3h", "xyy": "s3h", "yyy": "s3h"}


def _bcast(ap, n):
    """Repeat a [P, F] access pattern n times along a stride-0 middle dim."""
    dims = [list(d) for d in ap.ap]
    new = [dims[0], [0, n]] + dims[1:]
    return bass.AP(tensor=ap.tensor, offset=ap.offset, ap=new)


def _rep_segs(ap, seglen, k, n):
    """[P, k*seglen] contiguous -> [P, k, n, seglen]: each seglen-long
    segment repeated n times in sequence."""
    dims = [list(d) for d in ap.ap]
    new = [dims[0], [seglen, k], [0, n], [1, seglen]]
    return bass.AP(tensor=ap.tensor, offset=ap.offset, ap=new)


def _build():
    nc = bacc.Bacc(None, target_bir_lowering=False)

    pts_d = nc.declare_dram_parameter("pts", [3, NLOC], F32, False)
    w0_d = nc.declare_dram_parameter("W0f", [3, H], F32, False)
    w1_d = {s: nc.declare_dram_parameter(f"W1s_{s}", [H, H], F16, False)
            for s in STREAMS}
    wh_d = {(li, v): nc.declare_dram_parameter(f"Wh{li}_{v}", [H, H], F16,
                                               False)
            for li in range(2, 8) for v in VARIANTS}
    b_d = {li: nc.declare_dram_parameter(f"bb{li}", [H, 1], F32, False)
           for li in range(0, 8)}
    w8c_d = {v: nc.declare_dram_parameter(f"W8C_{v}", [H, 16 * 13], F16,
                                          False) for v in VARIANTS}
    b8_d = nc.declare_dram_parameter("b8v", [H, 1], F32, False)
    lam_d = nc.declare_dram_parameter("lam", [H, 4], F32, False)
    out_d = {k: nc.declare_dram_parameter(k, [H, PB], F32, True)
             for k in ["uo", "vo", "fuo", "fvo"]}

    with tile.TileContext(nc) as tc, ExitStack() as ctx:
        cpool = ctx.enter_context(tc.tile_pool(name="consts", bufs=1))
        apool = ctx.enter_context(tc.tile_pool(name="A", bufs=2))
        ipool = ctx.enter_context(tc.tile_pool(name="interm", bufs=1))
        zcp = ctx.enter_context(tc.tile_pool(name="zc", bufs=2))
        misc = ctx.enter_context(tc.tile_pool(name="misc", bufs=1))
        fpool = ctx.enter_context(tc.tile_pool(name="fin", bufs=1))
        zpool = ctx.enter_context(
            tc.tile_pool(name="psum_z", bufs=7, space="PSUM"))
        z8pool = ctx.enter_context(
            tc.tile_pool(name="psum_z8", bufs=1, space="PSUM"))

        def ctile(name, shape, dt):
            return cpool.tile(shape, dt, name=name, tag=name)

        # urgent consts (layer0/1) on the sync queue; the bulk of the
        # weights stream in from the otherwise-idle GpSimd queue
        w0s = ctile("w0s", [3, H], F32)
        nc.sync.dma_start(w0s[:], w0_d[:])
        bss = {}
        for li in range(0, 8):
            bss[li] = ctile(f"bs{li}", [H, 1], F32)
            nc.sync.dma_start(bss[li][:], b_d[li][:])
        w1s = {}
        for s in STREAMS:
            w1s[s] = ctile(f"w1s_{s}", [H, H], F16)
            nc.sync.dma_start(w1s[s][:], w1_d[s][:])
        whs = {}
        for li in range(2, 8):
            for v in VARIANTS:
                whs[(li, v)] = ctile(f"whs{li}_{v}", [H, H], F16)
                nc.gpsimd.dma_start(whs[(li, v)][:], wh_d[(li, v)][:])
        w8cs = {}
        for v in VARIANTS:
            w8cs[v] = ctile(f"w8cs_{v}", [H, 16 * 13], F16)
            nc.gpsimd.dma_start(w8cs[v][:], w8c_d[v][:])
        b8s = ctile("b8s", [H, 1], F32)
        nc.gpsimd.dma_start(b8s[:], b8_d[:])
        lams = ctile("lams", [H, 4], F32)
        nc.gpsimd.dma_start(lams[:], lam_d[:])

        z8stage = misc.tile([16, NLOC], F32, name="z8stage", tag="z8stage")

        V, G, S = nc.vector, nc.gpsimd, nc.scalar

        def new_tiles():
            t = {}
            for key, segs in GROUP_SEGS.items():
                t[key] = apool.tile([H, NCH * segs * CH], F16, name=key,
                                    tag=key)
            return t

        def seg(t, key, i, c):
            S_ = GROUP_SEGS[key]
            off = (c * S_ + i) * CH
            return t[:, off:off + CH]

        def chain_tiles():
            return {
                "t1": ipool.tile([H, BLK], F32, name="t1", tag="t1"),
                "s1": ipool.tile([H, BLK], F16, name="s1", tag="s1",
                                 bufs=2),
                "w3": ipool.tile([H, BLK], F16, name="w3", tag="w3"),
                "m": ipool.tile([H, BLK], F16, name="m", tag="m", bufs=2),
                "s3h": ipool.tile([H, BLK], F16, name="s3h", tag="s3h",
                                  bufs=2),
            }

        def chain_chunk(li, ct, s_t, c, zt):
            """tanh + derivative-chain scalars for chunk c from PSUM z."""
            csl = bass.ts(c, CH)
            S.activation(s_t[:, csl], zt[:], AF.Tanh, bias=bss[li][:])
            S.activation(ct["t1"][:, csl], s_t[:, csl], AF.Square)
            S.activation(ct["s1"][:, csl], ct["t1"][:, csl], AF.Copy,
                         bias=1.0, scale=-1.0)
            V.tensor_tensor(ct["m"][:, csl], s_t[:, csl], ct["s1"][:, csl],
                            OP.mult)
            # w3 = 6*t1 - 2 = 4 - 6*s1 (f16 input keeps the 4x DVE mode)
            V.tensor_scalar(ct["w3"][:, csl], ct["s1"][:, csl], -6.0, 4.0,
                            OP.mult, OP.add)
            V.tensor_tensor(ct["s3h"][:, csl], ct["w3"][:, csl],
                            ct["s1"][:, csl], OP.mult)

        def jet_elementwise(li, tiles):
            zcat = zcp.tile([H, NCH * 6 * CH], F16, name="zcat", tag="zcat")
            ct = chain_tiles()
            E = ipool.tile([H, NCH * 2 * CH], F16, name="E", tag="E")
            Ff = ipool.tile([H, NCH * 2 * CH], F16, name="Ff", tag="Ff")
            pcat = ipool.tile([H, NCH * 2 * CH], F16, name="pcat",
                              tag="pcat")
            s_t = tiles["s"]

            def zc_seg(i, c):
                off = (c * 6 + i) * CH
                return zcat[:, off:off + CH]

            ENG = {"V": V, "G": G}

            def z_consume(s, c, zt):
                csl = bass.ts(c, CH)
                if s == "v":
                    chain_chunk(li, ct, s_t, c, zt)
                elif s in ZSEG:
                    S.activation(zc_seg(ZSEG[s], c), zt[:], AF.Copy)
                elif ENG_TAIL == "D":
                    # stage z to SBUF via the DMA engines, multiply on
                    # GpSimd (which cannot read PSUM itself)
                    t32 = misc.tile([H, CH], F32, name="t32", tag="t32",
                                    bufs=4)
                    nc.sync.dma_start(t32[:], zt[:])
                    G.tensor_tensor(seg(tiles["T" + s], "T" + s, 0, c),
                                    ct["s1"][:, csl], t32[:], OP.mult)
                else:
                    ENG[ENG_TAIL].tensor_tensor(
                        seg(tiles["T" + s], "T" + s, 0, c),
                        ct["s1"][:, csl], zt[:], OP.mult)

            def chunk_lumps(c):
                csl = bass.ts(c, CH)
                zc_c = zcat[:, c * 6 * CH:(c + 1) * 6 * CH]
                e_c = E[:, c * 2 * CH:(c + 1) * 2 * CH]
                f_c = Ff[:, c * 2 * CH:(c + 1) * 2 * CH]
                p_c = pcat[:, c * 2 * CH:(c + 1) * 2 * CH]
                # G1 = s1 * [zx|zy|zt|zxx|zxy|zyy]
                V.tensor_tensor(tiles["G1"][:, c * 6 * CH:(c + 1) * 6 * CH],
                                _bcast(ct["s1"][:, csl], 6), zc_c, OP.mult)
                # E = [ex|ey] = m * [zx|zy]
                ENG[ENG_E].tensor_tensor(e_c, _bcast(ct["m"][:, csl], 2),
                                         zc_c[:, 0:2 * CH], OP.mult)
                # EXG = ex * [zx|zy|zt|zxx|zxy|zyy]
                V.tensor_tensor(tiles["EXG"][:, c * 6 * CH:(c + 1) * 6 * CH],
                                _bcast(e_c[:, 0:CH], 6), zc_c, OP.mult)
                # EYG = ey * [zy|zt|zxx|zxy|zyy]
                V.tensor_tensor(tiles["EYG"][:, c * 5 * CH:(c + 1) * 5 * CH],
                                _bcast(e_c[:, CH:2 * CH], 5),
                                zc_c[:, CH:6 * CH], OP.mult)
                # pcat = [zx^2|zy^2]
                if ENG_PCAT == "S":
                    S.activation(p_c, zc_c[:, 0:2 * CH], AF.Square)
                else:
                    ENG[ENG_PCAT].tensor_tensor(p_c, zc_c[:, 0:2 * CH],
                                                zc_c[:, 0:2 * CH], OP.mult)
                # F = [fx|fy] = s3h * pcat
                ENG[ENG_F].tensor_tensor(f_c, _bcast(ct["s3h"][:, csl], 2),
                                         p_c, OP.mult)
                # FG = [fx|fx|fy|fy] * [zx|zy|zx|zy]
                ENG[ENG_FG].tensor_tensor(
                    tiles["FG"][:, c * 4 * CH:(c + 1) * 4 * CH],
                    _rep_segs(f_c, CH, 2, 2),
                    _bcast(zc_c[:, 0:2 * CH], 2), OP.mult)

            return z_consume, chunk_lumps

        # ---------------- layers ----------------
        def layer0(blk):
            ptsb = misc.tile([3, BLK], F32, name="ptsb", tag="ptsb")
            nc.sync.dma_start(ptsb[:], pts_d[:, bass.ts(blk, BLK)])
            s_t = apool.tile([H, BLK], F16, name="s", tag="s")
            ct = chain_tiles()
            for c in range(NCH):
                zt = zpool.tile([H, CH], F32, name="z0", tag="z")
                nc.tensor.matmul(zt[:], w0s[:], ptsb[:, bass.ts(c, CH)],
                                 start=True, stop=True)
                chain_chunk(0, ct, s_t, c, zt)
            return {"s": s_t}, ct

        def hidden_matmuls(li, A_prev, z_consume, l0ct=None, extra=None):
            for c in range(NCH):
                for s in STREAMS:
                    zt = zpool.tile([H, CH], F32, name=f"z_{s}", tag="z")
                    if li == 1:
                        mov = (A_prev["s"][:, bass.ts(c, CH)]
                               if L1_MOV[s] == "s"
                               else l0ct[L1_MOV[s]][:, bass.ts(c, CH)])
                        nc.tensor.matmul(zt[:], w1s[s][:], mov,
                                         start=True, stop=True)
                    else:
                        adds = ADDENDS[s]
                        for j, (key, si, var) in enumerate(adds):
                            a = seg(A_prev[key], key, si, c)
                            nc.tensor.matmul(zt[:], whs[(li, var)][:], a,
                                             start=(j == 0),
                                             stop=(j == len(adds) - 1))
                    if extra is not None:
                        extra(s, c)
                    z_consume(s, c, zt)

        def hidden_layer(li, A_prev, l0ct=None, extra=None):
            tiles = new_tiles()
            z_consume, chunk_lumps = jet_elementwise(li, tiles)

            def consume_and_lump(s, c, zt):
                z_consume(s, c, zt)
                if s == STREAMS[-1]:
                    chunk_lumps(c)

            hidden_matmuls(li, A_prev, consume_and_lump, l0ct, extra)
            return tiles

        def layer8_emitter(blk, A_prev):
            """Returns extra(s, c): emits layer8(blk)'s matmuls for stream
            s interleaved into another layer's matmul stream."""
            total = sum(len(v) for v in ADDENDS.values())
            state = {}

            def extra(s, c):
                if s == "v":
                    state[c] = [z8pool.tile([16, CH], F32, name="z8",
                                            tag="z8"), 0]
                z8, _ = state[c]
                si = STREAMS.index(s)
                for (key, sj, var) in ADDENDS[s]:
                    k = state[c][1]
                    w8blk = w8cs[var][:, 16 * si:16 * si + 16]
                    nc.tensor.matmul(z8[:], w8blk,
                                     seg(A_prev[key], key, sj, c),
                                     start=(k == 0),
                                     stop=(k == total - 1))
                    state[c][1] += 1
                if s == STREAMS[-1]:
                    S.activation(z8stage[:, bass.ts(blk * NCH + c, CH)],
                                 z8[:], AF.Copy)

            return extra

        A8_prev = None
        for blk in range(NBLK):
            A, l0ct = layer0(blk)
            extra = (layer8_emitter(blk - 1, A8_prev)
                     if A8_prev is not None else None)
            for li in range(1, 8):
                A = hidden_layer(li, A, l0ct if li == 1 else None,
                                 extra if li == 1 else None)
            A8_prev = A
        # final block's layer8 runs standalone
        last_extra = layer8_emitter(NBLK - 1, A8_prev)
        for c in range(NCH):
            for s in STREAMS:
                last_extra(s, c)

        # ---------------- final fp32 jet -> outputs ----------------
        def ft(name):
            return fpool.tile([H, PB], F32, name=name, tag=name)

        def ftmp(name):
            return fpool.tile([H, PB], F32, name=name, tag="ftmp", bufs=6)

        Z = {}
        for si, s in enumerate(STREAMS):
            Z[s] = ft(f"Z_{s}")
            nc.sync.dma_start(Z[s][:], z8stage[si:si + 1, :])

        def tt(name, a, b, op=OP.mult, tmp=False):
            o = ftmp(name) if tmp else ft(name)
            V.tensor_tensor(o[:], a[:], b[:], op)
            return o

        def stt(name, a, sc, b, op0=OP.mult, op1=OP.mult, tmp=False):
            o = ftmp(name) if tmp else ft(name)
            V.scalar_tensor_tensor(o[:], a[:], sc, b[:], op0, op1)
            return o

        s8 = ft("s8")
        S.activation(s8[:], Z["v"][:], AF.Tanh, bias=b8s[:])
        t18 = ft("t18")
        S.activation(t18[:], s8[:], AF.Square)
        s18 = ft("s18")
        S.activation(s18[:], t18[:], AF.Copy, bias=1.0, scale=-1.0)
        w38 = ft("w38")
        S.activation(w38[:], t18[:], AF.Copy, bias=-1.0, scale=3.0)
        s2m8 = tt("s2m8", s8, s18)            # s2 = -2*s2m8
        s3h8 = tt("s3h8", w38, s18)           # s3 = 2*s3h8
        e8x = tt("e8x", s2m8, Z["x"])
        e8y = tt("e8y", s2m8, Z["y"])
        p8xx = ft("p8xx")
        S.activation(p8xx[:], Z["x"][:], AF.Square)
        p8yy = ft("p8yy")
        S.activation(p8yy[:], Z["y"][:], AF.Square)
        f8x = tt("f8x", s3h8, p8xx)
        f8y = tt("f8y", s3h8, p8yy)

        u = tt("u", s18, Z["y"])                      # u = p_y
        vv = stt("vv", s18, -1.0, Z["x"])             # v = -p_x

        def second(name, Ea, Zb, Zdd):
            a1 = stt(name + "_a", Ea, -2.0, Zb, tmp=True)
            a2 = tt(name + "_b", s18, Zdd, tmp=True)
            return tt(name, a1, a2, OP.add)

        p_xx = second("p_xx", e8x, Z["x"], Z["xx"])
        p_xy = second("p_xy", e8x, Z["y"], Z["xy"])
        p_yy = second("p_yy", e8y, Z["y"], Z["yy"])
        p_yt = second("p_yt", e8y, Z["t"], Z["yt"])
        mp_xt_a = stt("mp_xt_a", e8x, 2.0, Z["t"], tmp=True)
        mp_xt_b = stt("mp_xt_b", s18, -1.0, Z["xt"], tmp=True)
        mp_xt = tt("mp_xt", mp_xt_a, mp_xt_b, OP.add)  # -p_xt

        def third3(name, Fa, Za, Ea, Zaa, Zddd):
            a1 = stt(name + "_a", Fa, 2.0, Za, tmp=True)
            a2 = stt(name + "_b", Ea, -6.0, Zaa, tmp=True)
            a3 = tt(name + "_c", s18, Zddd, tmp=True)
            a12 = tt(name + "_ab", a1, a2, OP.add, tmp=True)
            return tt(name, a12, a3, OP.add)

        p_xxx = third3("p_xxx", f8x, Z["x"], e8x, Z["xx"], Z["xxx"])
        p_yyy = third3("p_yyy", f8y, Z["y"], e8y, Z["yy"], Z["yyy"])

        def third_m(name, Fa, Zb, Eb, Zaa, Ea, Zab, Zddd):
            # 2*Fa*Zb - 2*Eb*Zaa - 4*Ea*Zab + s1*Zddd
            a1 = stt(name + "_a", Fa, 2.0, Zb, tmp=True)
            a2 = stt(name + "_b", Eb, -2.0, Zaa, tmp=True)
            a3 = stt(name + "_c", Ea, -4.0, Zab, tmp=True)
            a4 = tt(name + "_d", s18, Zddd, tmp=True)
            a12 = tt(name + "_ab", a1, a2, OP.add, tmp=True)
            a34 = tt(name + "_cd", a3, a4, OP.add, tmp=True)
            return tt(name, a12, a34, OP.add)

        p_xxy = third_m("p_xxy", f8x, Z["y"], e8y, Z["xx"], e8x, Z["xy"],
                        Z["xxy"])
        p_xyy = third_m("p_xyy", f8y, Z["x"], e8x, Z["yy"], e8y, Z["xy"],
                        Z["xyy"])

        # f_u = p_yt + lam1*(u*p_xy + v*p_yy) - lam2*(p_xxy + p_yyy)
        fu_a = tt("fu_a", u, p_xy, tmp=True)
        fu_b = tt("fu_b", vv, p_yy, tmp=True)
        fu_ab = tt("fu_ab", fu_a, fu_b, OP.add, tmp=True)
        fu_l = stt("fu_l", fu_ab, lams[:, 0:1], p_yt, OP.mult, OP.add,
                   tmp=True)
        fu_c = tt("fu_c", p_xxy, p_yyy, OP.add, tmp=True)
        f_u = stt("f_u", fu_c, lams[:, 1:2], fu_l, OP.mult, OP.add)
        # f_v = -p_xt - lam1*(u*p_xx + v*p_xy) + lam2*(p_xxx + p_xyy)
        fv_a = tt("fv_a", u, p_xx, tmp=True)
        fv_b = tt("fv_b", vv, p_xy, tmp=True)
        fv_ab = tt("fv_ab", fv_a, fv_b, OP.add, tmp=True)
        fv_l = stt("fv_l", fv_ab, lams[:, 2:3], mp_xt, OP.mult, OP.add,
                   tmp=True)
        fv_c = tt("fv_c", p_xxx, p_xyy, OP.add, tmp=True)
        f_v = stt("f_v", fv_c, lams[:, 3:4], fv_l, OP.mult, OP.add)

        nc.sync.dma_start(out_d["uo"][:], u[:])
        nc.sync.dma_start(out_d["vo"][:], vv[:])
        nc.sync.dma_start(out_d["fuo"][:], f_u[:])
        nc.sync.dma_start(out_d["fvo"][:], f_v[:])

    return nc


_CACHE = {}


def _get_nc():
    if "nc" not in _CACHE:
        nc = _build()
        nc.finalize()
        _CACHE["nc"] = nc
    return _CACHE["nc"]


def make_inputs(inputs):
    """Host-side prep: full inputs -> per-core input maps."""
    f32 = np.float32
    x = np.asarray(inputs["x"], f32)[:, 0]
    y = np.asarray(inputs["y"], f32)[:, 0]
    t = np.asarray(inputs["t"], f32)[:, 0]
    pts = np.ascontiguousarray(np.stack([x, y, t], 0))          # [3, N]
    W0 = np.asarray(inputs["W0"], f32)
    cx, cy, ct = W0[0], W0[1], W0[2]
    w8 = np.asarray(inputs["W8"], f32)[:, 0]
    scales = {0: 1.0, 2: -2.0, 4: -4.0, 6: -6.0}
    shared = {
        "W0f": np.ascontiguousarray(W0),
        "b8v": np.full([H, 1], np.asarray(inputs["b8"]).reshape(-1)[0], f32),
    }
    # layer1 stationaries with layer0 jet constants folded in
    c0 = {"v": np.ones(H, f32), "x": cx, "y": cy, "t": ct,
          "xx": -2 * cx * cx, "xy": -2 * cx * cy, "yy": -2 * cy * cy,
          "xt": -2 * cx * ct, "yt": -2 * cy * ct,
          "xxx": cx ** 3, "xxy": cx * cx * cy, "xyy": cx * cy * cy,
          "yyy": cy ** 3}
    W1 = np.asarray(inputs["W1"], f32)
    for s in STREAMS:
        shared[f"W1s_{s}"] = (W1 * c0[s][:, None]).astype(np.float16)
    for v, sc in scales.items():
        W8C = np.zeros([H, 16 * 13], np.float16)
        for s in range(13):
            W8C[:, 16 * s + s] = (w8 * sc).astype(np.float16)
        shared[f"W8C_{v}"] = W8C
    lam1 = f32(np.asarray(inputs["lam1"]).reshape(-1)[0])
    lam2 = f32(np.asarray(inputs["lam2"]).reshape(-1)[0])
    shared["lam"] = np.tile(np.array([[lam1, -lam2, -lam1, lam2]], f32),
                            (H, 1))
    for li in range(2, 8):
        Wf = np.asarray(inputs[f"W{li}"], f32)
        for v, sc in scales.items():
            shared[f"Wh{li}_{v}"] = (Wf * sc).astype(np.float16)
    for li in range(0, 8):
        shared[f"bb{li}"] = np.asarray(
            inputs[f"b{li}"], f32).reshape(H, 1).copy()

    in_maps = []
    for c in range(N_CORES):
        m = dict(shared)
        m["pts"] = np.ascontiguousarray(pts[:, c * NLOC:(c + 1) * NLOC])
        in_maps.append(m)
    return in_maps


def kernel(**inputs):
    nc = _get_nc()
    f32 = np.float32
    in_maps = make_inputs(inputs)
    trace = bool(os.environ.get("BASS_KERNEL_TRACE"))
    tdir = os.environ.get("BASS_KERNEL_TRACE_DIR") or None
    res = run_bass_kernel_spmd(nc, in_maps, list(range(N_CORES)),
                               trace=trace, tmpdir=tdir)
    kernel.last_exec_time_ns = res.exec_time_ns
    outs = []
    for name in ["uo", "vo", "fuo", "fvo"]:
        full = np.concatenate(
            [np.asarray(res.results[c][name], f32).reshape(-1)
             for c in range(N_CORES)])
        outs.append(full[:, None])
    return tuple(outs)


kernel.last_exec_time_ns = None
